# revision 10
# baseline (speedup 1.0000x reference)
"""Multi-head attention (B=4, T=2048, D=1024, H=16) on 8 NeuronCores.

Sharding: core c handles batch b=c//2 and head-group hg=c%2 (8 heads).
Per core: qk^T projection (transposed layout), v projection (natural),
scores-transposed attention with exp (no max subtraction; |scores| < ~3),
rowsum via a ones-column appended to v, out-projection from o^T.
Host sums the two tensor-parallel partials per batch and adds b_out.

dtypes: projections in f32r (fp32 with 12-bit mantissa, full PE speed),
attention internals (q/k/v/p) in bf16, accumulation fp32.
"""
import sys

sys.path.insert(0, "/opt/trn_rl_repo")

import numpy as np

T = 2048
D = 1024
NH = 16
DH = 64
TK = T // 128          # 16 t-tiles
KT = D // 128          # 8 contraction tiles
HL = NH // 2           # 8 heads per core
MC = 512               # m-chunk width
NMC = T // MC          # 4 chunks
SCALE = 1.0 / np.sqrt(DH)

_PROGRAM = None


def _round_f32r(x):
    """Round fp32 to f32r (round-to-nearest, 12-bit mantissa)."""
    bits = np.ascontiguousarray(x, dtype=np.float32).view(np.uint32).astype(np.uint64)
    bits = (bits + 0x400 + ((bits >> 11) & 1)) & 0xFFFFF800
    return bits.astype(np.uint32).view(np.float32)


def _build_program():
    import os
    import concourse.bacc as bacc
    import concourse.tile as tile
    from concourse import mybir

    kphase = os.environ.get("KPHASE", "ABC")

    f32 = mybir.dt.float32
    f32r = mybir.dt.float32r
    bf16 = mybir.dt.bfloat16
    EXP = mybir.ActivationFunctionType.Exp

    nc = bacc.Bacc("TRN2", target_bir_lowering=False)

    xT = nc.dram_tensor("xT", [D, T], f32r, kind="ExternalInput")
    wqk = nc.dram_tensor("wqk", [D, D], f32r, kind="ExternalInput")
    wv = nc.dram_tensor("wv", [D, 512], f32r, kind="ExternalInput")
    wout = nc.dram_tensor("wout", [512, D], f32r, kind="ExternalInput")
    bqk = nc.dram_tensor("bqk", [D, 1], f32, kind="ExternalInput")
    bvb = nc.dram_tensor("bvb", [1, 512], f32, kind="ExternalInput")
    out_p = nc.dram_tensor("out_p", [T, D], f32, kind="ExternalOutput")
    # scratch for rowsum-reciprocal partition broadcast (SBUF sources cannot
    # have zero partition step in DMA; DRAM sources can)
    rscr = nc.dram_tensor("rscr", [4, NMC, 2, MC], f32)

    with tile.TileContext(nc) as tc:
        with (
            tc.tile_pool(name="persist", bufs=1) as persist,
            tc.tile_pool(name="small", bufs=4) as small,
        ):
            # persistent SBUF tensors
            qkT = [persist.tile([128, T], bf16, tag=f"qk{j}", name=f"qk{j}")
                   for j in range(KT)]
            vsb = [persist.tile([128, HL, DH + 1], bf16, tag=f"v{t}",
                                name=f"v{t}") for t in range(TK)]
            oT = [persist.tile([128, T], f32r, tag=f"ot{p}", name=f"ot{p}")
                  for p in range(4)]
            bqk_sb = [persist.tile([128, 1], f32, tag=f"bq{k}", name=f"bq{k}")
                      for k in range(KT)]
            bv_bc = persist.tile([128, 512], f32, tag="bvbc", name="bvbc")

            nc.sync.dma_start(out=bv_bc[:], in_=bvb[0:1, :].to_broadcast([128, 512]))
            for k in range(KT):
                nc.sync.dma_start(out=bqk_sb[k][:],
                                  in_=bqk[128 * k:128 * (k + 1), 0:1])

            # ---------------- Phase A: projections ----------------
            with tc.tile_pool(name="xtp", bufs=1) as xtp:
                xT_sb = [xtp.tile([128, T], f32r, tag=f"xt{k}", name=f"xt{k}")
                         for k in range(KT)]
                for k in range(KT):
                    nc.sync.dma_start(out=xT_sb[k][:],
                                      in_=xT[128 * k:128 * (k + 1), :])

                # A1: v (natural layout [t, j]), bias added, ones col installed
                with tc.tile_pool(name="wvp", bufs=1) as wvp:
                    wv_sb = [wvp.tile([128, 512], f32r, tag=f"wv{k}",
                                      name=f"wv{k}") for k in range(KT)]
                    for k in range(KT):
                        nc.sync.dma_start(out=wv_sb[k][:],
                                          in_=wv[128 * k:128 * (k + 1), :])
                    with tc.tile_pool(name="vps", bufs=4, space="PSUM") as vps:
                        for t in range(TK):
                            ps = vps.tile([128, 512], f32, tag="vps", name="vps")
                            for k in range(KT):
                                nc.tensor.matmul(
                                    ps[:],
                                    xT_sb[k][:, 128 * t:128 * (t + 1)],
                                    wv_sb[k][:],
                                    start=(k == 0), stop=(k == KT - 1))
                            nc.vector.tensor_add(
                                vsb[t][:, :, 0:DH],
                                ps[:].rearrange("p (h d) -> p h d", h=HL),
                                bv_bc[:].rearrange("p (h d) -> p h d", h=HL))
                            nc.vector.memset(vsb[t][:, :, DH:DH + 1], 1.0)

                # A2: q,k transposed layout [j, t]
                with tc.tile_pool(name="wqkp", bufs=1) as wqkp:
                    wqk_sb = [wqkp.tile([128, D], f32r, tag=f"wq{k}",
                                        name=f"wq{k}") for k in range(KT)]
                    for k in range(KT):
                        nc.sync.dma_start(out=wqk_sb[k][:],
                                          in_=wqk[128 * k:128 * (k + 1), :])
                    with tc.tile_pool(name="qkps", bufs=4, space="PSUM") as qkps:
                        for j in range(KT):
                            for c in range(NMC):
                                ps = qkps.tile([128, MC], f32, tag="qkps",
                                               name="qkps")
                                for k in range(KT):
                                    nc.tensor.matmul(
                                        ps[:],
                                        wqk_sb[k][:, 128 * j:128 * (j + 1)],
                                        xT_sb[k][:, MC * c:MC * (c + 1)],
                                        start=(k == 0), stop=(k == KT - 1))
                                nc.vector.tensor_scalar_add(
                                    qkT[j][:, MC * c:MC * (c + 1)],
                                    ps[:], bqk_sb[j][:])

            # ---------------- Phase B: attention ----------------
            if "B" not in kphase:
                with tc.tile_pool(name="dbg", bufs=1) as dbg:
                    dt = dbg.tile([128, 512], f32, tag="dbg", name="dbg")
                    nc.vector.tensor_copy(dt[:], qkT[0][:, 0:512])
                    nc.sync.dma_start(out=out_p[0:128, 0:512], in_=dt[:])
            with (
                tc.tile_pool(name="ptp", bufs=2) as ptp,
                tc.tile_pool(name="rbp", bufs=4) as rbp,
                tc.tile_pool(name="tmpb", bufs=2) as tmpb,
                tc.tile_pool(name="sps", bufs=2, space="PSUM") as sps,
                tc.tile_pool(name="ops", bufs=1, space="PSUM") as ops,
            ):
                for hp in range(4 if "B" in kphase else 0):
                    qt = qkT[hp]
                    kt = qkT[4 + hp]
                    hA, hB = 2 * hp, 2 * hp + 1
                    for mc in range(NMC):
                        ptA = ptp.tile([128, TK, MC], bf16, tag="ptA", name="ptA")
                        ptB = ptp.tile([128, TK, MC], bf16, tag="ptB", name="ptB")
                        for nt in range(TK):
                            sA = sps.tile([128, MC], f32, tag="sA", name="sA")
                            sB = sps.tile([128, MC], f32, tag="sB", name="sB")
                            nc.tensor.matmul(
                                sA[:], kt[0:64, 128 * nt:128 * (nt + 1)],
                                qt[0:64, MC * mc:MC * (mc + 1)],
                                start=True, stop=True, tile_position=(0, 0))
                            nc.tensor.matmul(
                                sB[:], kt[64:128, 128 * nt:128 * (nt + 1)],
                                qt[64:128, MC * mc:MC * (mc + 1)],
                                start=True, stop=True, tile_position=(64, 0))
                            nc.scalar.activation(ptA[:, nt, :], sA[:], EXP,
                                                 scale=float(SCALE))
                            nc.scalar.activation(ptB[:, nt, :], sB[:], EXP,
                                                 scale=float(SCALE))
                        oA = ops.tile([65, MC], f32, tag="oA", name="oA")
                        oB = ops.tile([65, MC], f32, tag="oB", name="oB")
                        for nt in range(TK):
                            nc.tensor.matmul(oA[:], vsb[nt][:, hA, :],
                                             ptA[:, nt, :],
                                             start=(nt == 0), stop=(nt == TK - 1))
                            nc.tensor.matmul(oB[:], vsb[nt][:, hB, :],
                                             ptB[:, nt, :],
                                             start=(nt == 0), stop=(nt == TK - 1))
                        # rowsums sit in psum partition 64; recip in-lane then
                        # DMA-broadcast to partitions 0-63
                        rA = rbp.tile([128, MC], f32, tag="rA", name="rA")
                        rB = rbp.tile([128, MC], f32, tag="rB", name="rB")
                        nc.vector.reciprocal(rA[64:65, :], oA[64:65, :])
                        nc.vector.reciprocal(rB[64:65, :], oB[64:65, :])
                        nc.sync.dma_start(out=rscr[hp, mc, 0, :],
                                          in_=rA[64:65, :])
                        nc.sync.dma_start(out=rscr[hp, mc, 1, :],
                                          in_=rB[64:65, :])
                        nc.sync.dma_start(
                            out=rA[0:64, :],
                            in_=rscr[hp, mc, 0:1, :].to_broadcast([64, MC]))
                        nc.sync.dma_start(
                            out=rB[0:64, :],
                            in_=rscr[hp, mc, 1:2, :].to_broadcast([64, MC]))
                        # normalize: head A direct; head B via tmp + DMA shift
                        nc.vector.tensor_mul(
                            oT[hp][0:64, MC * mc:MC * (mc + 1)],
                            oA[0:64, :], rA[0:64, :])
                        tB = tmpb.tile([64, MC], f32r, tag="tB", name="tB")
                        nc.vector.tensor_mul(tB[:], oB[0:64, :], rB[0:64, :])
                        nc.sync.dma_start(
                            out=oT[hp][64:128, MC * mc:MC * (mc + 1)],
                            in_=tB[:])

            # ---------------- Phase C: output projection ----------------
            with (
                tc.tile_pool(name="wop", bufs=1) as wop,
                tc.tile_pool(name="cop", bufs=3) as cop,
                tc.tile_pool(name="cps", bufs=2, space="PSUM") as cps,
            ):
                nco = 4 if ("C" in kphase and "B" in kphase) else 0
                wout_sb = [wop.tile([128, D], f32r, tag=f"wo{j}", name=f"wo{j}")
                           for j in range(nco)]
                for j in range(nco):
                    nc.sync.dma_start(out=wout_sb[j][:],
                                      in_=wout[128 * j:128 * (j + 1), :])
                for t in range(TK if "C" in kphase and "B" in kphase else 0):
                    for ch in range(2):
                        ps = cps.tile([128, 512], f32, tag="cps", name="cps")
                        for j in range(4):
                            nc.tensor.matmul(
                                ps[:],
                                oT[j][:, 128 * t:128 * (t + 1)],
                                wout_sb[j][:, 512 * ch:512 * (ch + 1)],
                                start=(j == 0), stop=(j == 3))
                        ot = cop.tile([128, 512], f32, tag="co", name="co")
                        nc.vector.tensor_copy(ot[:], ps[:])
                        nc.sync.dma_start(
                            out=out_p[128 * t:128 * (t + 1),
                                      512 * ch:512 * (ch + 1)],
                            in_=ot[:])

    nc.compile()
    return nc


def _get_program():
    global _PROGRAM
    if _PROGRAM is None:
        _PROGRAM = _build_program()
    return _PROGRAM


def _make_in_maps(x, w_qkv, b_qkv, w_out):
    in_maps = []
    for c in range(8):
        b, hg = c // 2, c % 2
        qs, ks, vs = 512 * hg, D + 512 * hg, 2 * D + 512 * hg
        in_maps.append({
            "xT": _round_f32r(x[b].T),
            "wqk": _round_f32r(np.concatenate(
                [w_qkv[:, qs:qs + 512], w_qkv[:, ks:ks + 512]], axis=1)),
            "wv": _round_f32r(w_qkv[:, vs:vs + 512]),
            "wout": _round_f32r(w_out[512 * hg:512 * hg + 512, :]),
            "bqk": np.ascontiguousarray(np.concatenate(
                [b_qkv[qs:qs + 512], b_qkv[ks:ks + 512]])[:, None],
                dtype=np.float32),
            "bvb": np.ascontiguousarray(b_qkv[vs:vs + 512][None, :],
                                        dtype=np.float32),
        })
    return in_maps


def kernel(x, w_qkv, b_qkv, w_out, b_out, _trace=False):
    from concourse.bass_utils import run_bass_kernel_spmd

    x = np.asarray(x, dtype=np.float32)
    w_qkv = np.asarray(w_qkv, dtype=np.float32)
    b_qkv = np.asarray(b_qkv, dtype=np.float32)
    w_out = np.asarray(w_out, dtype=np.float32)
    b_out = np.asarray(b_out, dtype=np.float32)

    nc = _get_program()
    in_maps = _make_in_maps(x, w_qkv, b_qkv, w_out)
    kres = run_bass_kernel_spmd(nc, in_maps, list(range(8)), trace=_trace)
    res = kres.results

    B = x.shape[0]
    out = np.empty((B, T, D), dtype=np.float32)
    for b in range(B):
        out[b] = res[2 * b]["out_p"] + res[2 * b + 1]["out_p"] + b_out
    if _trace:
        return out, kres
    return out
